# revision 62
# baseline (speedup 1.0000x reference)
"""CQAttention Trainium2 Bass kernel.

Computes, per batch b (B=128, D=128, LC=400, LQ=50):
    S = Wc.C (over rows) + Wq.Q (over cols) + Wqc.(C*Q)   [LC, LQ]
    S1 = softmax(S, axis=LQ); S2 = softmax(S, axis=LC)
    A  = Q @ S1^T                    [D, LC]
    Bm = (C @ S2) @ S1^T             [D, LC]
    out = concat([C, A, C*A, C*Bm])  [4D, LC]

Sharding: data-parallel over batch, 16 batches per core x 8 cores.

Kernel math (per batch, ST = S^T layout [LQ=50 part, LC=400 free]):
    QW[d,j]  = Wqc[d]*Q[d,j] + Wc[d]                (2-scalar op)
    ST'      = QW^T @ C                 [50,400]    (folds the Wc.C row term)
    cT[j]    = Q^T @ Wq                 [50,1]
    expST    = exp(ST' + cT)  (+row sums den2)      (ACT, accum_out)
    d1b      = ones[50,128]^T @ expST   [128,400]   (bcast column sums over j)
    R        = recip(d1b); CR = C*R                 (softmax-over-j deferred
               to the output muls: keeps d1b/recip off the critical chain)
    expS/CT  = PE transposes of expST / C; chunks 0-2 full 128-wide, chunk 3
               is the 16 real trailing rows (no zero padding anywhere)
    T1T_raw  = sum_c expS_c^T @ CT_c    [50,128]
    T1T      = T1T_raw * recip(den2)                (softmax over i, folded)
    A_u      = QT^T @ expST             [128,400]   (unnormalized)
    Bm_u     = T1T^T @ expST            [128,400]
    out rows: [C, R*A_u, CR*A_u, CR*Bm_u]

Schedule: two-stage software pipeline (FRONT(b+1) issues before BACK(b))
so exp(b+1) stays ahead of t1t_sb(b) in the in-order ACT queue; C/Q loads
issue on the Pool SWDGE queue (no HWDGE contention, no head-of-line
blocking behind waiting stores); stores on SP. C/Q rings are f32r-typed
so edge batches (head_fast/tail_fast), which run at mid p-state, can use
f32r matmuls/transposes and a bf16 T1 path to halve the latency chain
where the PE is not streaming.

PSUM layout (8 banks, all pools depth 2):
    pp_st  x2: [50,404]  stp(0:400)+ctp(400:401); d1b overwrites stp after exp
    pp_a   x2: [128,512] ctT chunks 0-2 (0:384) + qtp (50p, 384:512)
    pp_b   x2: [128,456] esp (0:200) + ctT3 (16p, 200:328) + t1tp (50p, 328:456)
    pp_ab  x1: [128,1024] A (0:400) + Bm (512:912), two banks
"""

import os
import sys
import time

# The kernel executes via the axon PJRT backend; make sure it isn't masked
# by an explicit cpu pin (harmless if jax is already initialized with axon).
_jp = os.environ.get("JAX_PLATFORMS", "")
if _jp and "axon" not in _jp:
    os.environ["JAX_PLATFORMS"] = "axon," + _jp

for _p in ("/opt/trn_rl_repo", "/root/.axon_site/_ro/trn_rl_repo"):
    if _p not in sys.path:
        sys.path.append(_p)

import numpy as np

B, D, LC, LQ = 128, 128, 400, 50
N_CORES = 8
BPC = B // N_CORES  # 16 batches per core
LC3 = 3 * 128       # columns covered by the three full transpose chunks
LCR = LC - LC3      # 16 trailing rows handled by the narrow chunk

USE_F32R_BIG = True


def build_nc(bpc=BPC, use_f32r_big=USE_F32R_BIG, enable_asserts=False,
             mid_bufs=7, outp_bufs=6, io_bufs=5, qw_pool_engine=True,
             c_halves=5, e_slots=5, ab_bufs=1, st_bufs=2, a_bufs=2, b_bufs=2,
             t1t_bf16=False, stp_f32r=False, tr_f32r=False,
             s1t_on_pool=False, o1_on_act=False, o1_split=False,
             loads_on_pool=False, load_group=2, split_o23=False,
             split_store=0, loads_on="sp", cstore_on="sp", cstore_back=True,
             warm_sp=False, pipe_depth=2, fold_den1=True, cr_on_pool=True,
             head_fast=0, tail_fast=99, o23_tail=0, o1act_tail=0,
             es_f32r=False, c_f32r=True, stp_tail=99, w_on_act=False,
             t1tdve_tail=0, warm_act=False, d2d_cstore=0, d2d_on_sp=False,
             a_first=False, sbb_dve_tail=0, sba_dve_tail=0,
             tr_tail=99, t1t_tail=99, stp_head=0, t1t_head=0,
             pool_alloc_mode="stack", detect_races=True):
    import concourse.bacc as bacc
    import concourse.tile as tile
    from concourse import mybir
    from concourse.masks import make_identity

    F32 = mybir.dt.float32
    F32R = mybir.dt.float32r
    BF16 = mybir.dt.bfloat16
    AFT = mybir.ActivationFunctionType
    ALU = mybir.AluOpType

    FR_BIG = F32R if use_f32r_big else F32
    T1DT = BF16 if t1t_bf16 else F32

    assert bpc % 2 == 0
    assert load_group == 2
    nc = bacc.Bacc("TRN2", target_bir_lowering=False, debug=False,
                   enable_asserts=enable_asserts, num_devices=N_CORES,
                   detect_race_conditions=detect_races)
    C_ap = nc.dram_tensor("C", [bpc, D, LC], F32, kind="ExternalInput").ap()
    Q_ap = nc.dram_tensor("Q", [bpc, D, LQ], F32, kind="ExternalInput").ap()
    W_ap = nc.dram_tensor("W", [bpc, 1, 3 * D], F32, kind="ExternalInput").ap()
    out_ap = nc.dram_tensor("out", [bpc, 4 * D, LC], F32,
                            kind="ExternalOutput").ap()

    with tile.TileContext(nc, pool_alloc_mode=pool_alloc_mode) as tc:
        from contextlib import ExitStack
        with ExitStack() as ctx:
            consts = ctx.enter_context(tc.tile_pool(name="consts", bufs=1))
            io = ctx.enter_context(tc.tile_pool(name="io", bufs=io_bufs))
            mid = ctx.enter_context(tc.tile_pool(name="mid", bufs=mid_bufs))
            outp = ctx.enter_context(tc.tile_pool(name="outp", bufs=outp_bufs))
            pp_st = ctx.enter_context(
                tc.tile_pool(name="pp_st", bufs=st_bufs, space="PSUM"))
            pp_a = ctx.enter_context(
                tc.tile_pool(name="pp_a", bufs=a_bufs, space="PSUM"))
            pp_b = ctx.enter_context(
                tc.tile_pool(name="pp_b", bufs=b_bufs, space="PSUM"))
            pp_ab = ctx.enter_context(
                tc.tile_pool(name="pp_ab", bufs=ab_bufs, space="PSUM"))

            # --- constants ---
            ident = consts.tile([128, 128], F32)
            make_identity(nc, ident)
            ident_r_t = consts.tile([128, 128], F32R)
            nc.vector.tensor_copy(ident_r_t, ident)
            ident_r = ident_r_t[:]
            ones_f32 = consts.tile([LQ, 128], F32)
            nc.vector.memset(ones_f32, 1.0)
            onesmat = consts.tile([LQ, 128], FR_BIG)
            nc.vector.tensor_copy(onesmat, ones_f32)

            # --- W preload: [bpc,384] -> per-d columns [128, 3*bpc] ---
            w_stage = consts.tile([bpc, 3 * D], F32)
            (nc.scalar if w_on_act else nc.sync).dma_start(
                w_stage, W_ap[:, 0, :])
            wTp = pp_a.tile([128, 3 * bpc], F32, tag="a")
            for k in range(3):
                nc.tensor.matmul(
                    wTp[:, k * bpc:(k + 1) * bpc],
                    w_stage[:, k * D:(k + 1) * D],
                    ident[:bpc, :bpc],
                    is_transpose=True, start=True, stop=True)
            w_all = consts.tile([128, 3 * bpc], F32)
            nc.vector.tensor_copy(w_all, wTp)

            # C passthrough rows for the first few batches go as cold
            # DRAM->DRAM copies on the ACT queue: they have no deps, so the
            # transfer lands in the startup hole of the DMA stream.
            for _b in range(d2d_cstore):
                (nc.sync if d2d_on_sp else nc.scalar).dma_start(
                    out_ap[_b, 0:D, :], C_ap[_b, :, :])

            # Manually double-buffered C-pair and expST rings (no pad columns
            # needed: the 4th transpose chunk is 16 partitions wide).
            CDT = F32R if c_f32r else F32
            cbuf = consts.tile([D, c_halves * 2 * LC], CDT)
            ebuf_f32 = consts.tile([LQ, e_slots * LC], F32)
            ebuf = ebuf_f32[:].bitcast(FR_BIG) if FR_BIG != F32 else ebuf_f32[:]

            def f32v(ap):
                return ap.bitcast(F32) if ap.dtype == F32R else ap

            # Two-stage software pipeline: FRONT(b) produces everything up
            # to the merged SBUF copies; BACK(b) (issued after FRONT(b+1))
            # runs t1t/A/Bm/output ops. This keeps exp(b+1) ahead of
            # t1t_sb(b) in the ACT queue, breaking the per-batch
            # ACT->PE->ACT round-trip that otherwise sets the initiation
            # interval.
            qpair = None
            stash = {}

            def front(b):
                nonlocal qpair
                # edge batches run at mid p-state (PE not continuously
                # busy), where f32r/bf16 matmuls halve the latency chain
                fast = (b < head_fast) or (b >= tail_fast)
                b_stp_f32r = (stp_f32r or fast or b >= stp_tail
                              or b < stp_head) and c_f32r
                b_tr_f32r = (fast or b >= tr_tail) and c_f32r
                b_es_f32r = (es_f32r or fast) and FR_BIG == F32R
                b_t1t_bf16 = (t1t_bf16 or fast or b >= t1t_tail
                              or b < t1t_head)
                b_t1dt = BF16 if b_t1t_bf16 else F32R
                wq_col = w_all[:, b:b + 1]
                wc_col = w_all[:, bpc + b:bpc + b + 1]
                wqc_col = w_all[:, 2 * bpc + b:2 * bpc + b + 1]

                k = b % 2
                half = (b // 2) % c_halves
                cpair = cbuf[:, half * 2 * LC:(half + 1) * 2 * LC]
                if b % load_group == 0:
                    g = load_group
                    ld = {"sp": nc.sync, "act": nc.scalar,
                          "pool": nc.gpsimd}[loads_on]
                    if loads_on_pool:
                        ld = nc.gpsimd
                    dst = cbuf[:, half * 2 * LC:half * 2 * LC + g * LC]
                    qpair = io.tile([D, g * LQ], CDT, tag="qpair")
                    if b == 0 and warm_sp:
                        # cold start: single-batch C first on the fast HWDGE
                        # path so exp(0) fires as early as possible; with
                        # warm_act, C0/Q0 issue on the ACT queue in parallel
                        # with the W load on SP
                        w0 = nc.scalar if warm_act else nc.sync
                        w0.dma_start(dst[:, :LC],
                                     C_ap[0, :, :].bitcast(CDT))
                        w0.dma_start(
                            qpair[:].rearrange("p (t s) -> p t s", t=g),
                            Q_ap[0:g].rearrange("t d j -> d t j").bitcast(CDT))
                        nc.sync.dma_start(dst[:, LC:2 * LC],
                                          C_ap[1, :, :].bitcast(CDT))
                    else:
                        ld.dma_start(
                            dst.rearrange("p (t s) -> p t s", t=g),
                            C_ap[b:b + g].rearrange("t d i -> d t i").bitcast(CDT))
                        ld.dma_start(
                            qpair[:].rearrange("p (t s) -> p t s", t=g),
                            Q_ap[b:b + g].rearrange("t d j -> d t j").bitcast(CDT))
                ct = cpair[:, k * LC:(k + 1) * LC]       # [128, 400]
                qt = qpair[:, (b % load_group) * LQ:
                           (b % load_group + 1) * LQ]    # [128, 50]

                if not cstore_back:
                    nc.sync.dma_start(out_ap[b, 0:D, :], f32v(ct))

                # QW = Wqc*Q + Wc  (two-scalar op)
                qw = mid.tile([D, LQ], F32R if b_stp_f32r else F32,
                              tag="qw")
                qw_eng = nc.gpsimd if qw_pool_engine else nc.vector
                qw_eng.tensor_scalar(qw, f32v(qt), wqc_col, wc_col,
                                     ALU.mult, ALU.add)

                # ST' = QW^T @ C [50,400]; cT = Q^T @ Wq packed into the
                # spare bytes of the same PSUM bank
                stb = pp_st.tile([128, LC + 4], F32, tag="st", name="stb")
                stp = stb[:LQ, :LC]
                if b_stp_f32r:
                    nc.tensor.matmul(stp, qw, ct, start=True, stop=True)
                else:
                    nc.tensor.matmul(stp, qw, f32v(ct),
                                     start=True, stop=True)
                ctp = stb[:LQ, LC:LC + 1]
                nc.tensor.matmul(ctp, f32v(qt), wq_col,
                                 start=True, stop=True)
                ct_sb = mid.tile([LQ, 1], F32, tag="ctsb")
                nc.vector.tensor_copy(ct_sb, ctp)

                # QT transpose [50,128] into pp_a cols 384:512; C^T chunks
                # 0-2 into cols 0:384; the 16-row chunk 3 into pp_b.
                pa = pp_a.tile([128, 512], F32, tag="a", name="pa")
                pb_t = pp_b.tile([128, 456], F32, tag="b", name="pb")
                if b_tr_f32r:
                    nc.tensor.matmul(pa[:LQ, LC3:].bitcast(F32R),
                                     qt, ident_r,
                                     is_transpose=True, start=True, stop=True)
                    for c in range(3):
                        nc.tensor.matmul(
                            pa[:, c * 128:(c + 1) * 128].bitcast(F32R),
                            ct[:, c * 128:(c + 1) * 128],
                            ident_r, is_transpose=True, start=True, stop=True)
                    nc.tensor.matmul(
                        pb_t[:LCR, 200:328].bitcast(F32R),
                        ct[:, LC3:], ident_r,
                        is_transpose=True, start=True, stop=True)
                else:
                    nc.tensor.matmul(pa[:LQ, LC3:], f32v(qt), ident,
                                     is_transpose=True, start=True, stop=True)
                    for c in range(3):
                        nc.tensor.matmul(
                            pa[:, c * 128:(c + 1) * 128],
                            f32v(ct[:, c * 128:(c + 1) * 128]), ident,
                            is_transpose=True, start=True, stop=True)
                    nc.tensor.matmul(pb_t[:LCR, 200:328], f32v(ct[:, LC3:]),
                                     ident,
                                     is_transpose=True, start=True, stop=True)

                # expST = exp(ST' + cT), den2 = row sums
                eslot = b % e_slots
                expst = ebuf[:, eslot * LC:(eslot + 1) * LC]
                den2 = mid.tile([LQ, 1], F32, tag="den2")
                nc.scalar.activation(expst, stp, AFT.Exp, bias=ct_sb,
                                     accum_out=den2)
                r2 = mid.tile([LQ, 1], F32, tag="r2")
                nc.vector.reciprocal_approx_fast(r2, den2)

                # den1 broadcast: d1b[p,i] = sum_j expST[j,i]; overwrites stp
                # (exp is done). With fold_den1 the broadcast covers all 128
                # partitions so the recip can scale the A/Bm outputs directly.
                if fold_den1:
                    d1b = stb[:, :LC]
                    nc.tensor.matmul(d1b, onesmat, expst,
                                     start=True, stop=True)
                else:
                    d1b = stb[:LQ, :LC]
                    nc.tensor.matmul(d1b, onesmat[:, :LQ], expst,
                                     start=True, stop=True)

                # expS^T chunks into pp_b cols 0:200 (chunk 3 is 16 rows)
                if b_es_f32r:
                    for c in range(3):
                        es_in = expst[:, c * 128:(c + 1) * 128]
                        if es_in.dtype != F32R:
                            es_in = es_in.bitcast(F32R)
                        nc.tensor.matmul(
                            pb_t[:, c * LQ:(c + 1) * LQ].bitcast(F32R),
                            es_in, ident_r[:LQ, :LQ],
                            is_transpose=True, start=True, stop=True)
                    es3 = expst[:, LC3:]
                    if es3.dtype != F32R:
                        es3 = es3.bitcast(F32R)
                    nc.tensor.matmul(pb_t[:LCR, 3 * LQ:4 * LQ].bitcast(F32R),
                                     es3, ident_r[:LQ, :LQ],
                                     is_transpose=True, start=True, stop=True)
                else:
                    for c in range(3):
                        nc.tensor.matmul(
                            pb_t[:, c * LQ:(c + 1) * LQ],
                            f32v(expst[:, c * 128:(c + 1) * 128]),
                            ident[:LQ, :LQ],
                            is_transpose=True, start=True, stop=True)
                    nc.tensor.matmul(pb_t[:LCR, 3 * LQ:4 * LQ],
                                     f32v(expst[:, LC3:]), ident[:LQ, :LQ],
                                     is_transpose=True, start=True, stop=True)

                if fold_den1:
                    # R = 1/den1 over all 128 partitions; CR = C*R. The A/Bm
                    # matmuls then use raw expST as rhs (softmax-over-j is
                    # applied at the output muls), which drops d1b->r1b->s1t
                    # from the inter-batch critical chain.
                    rden = mid.tile([128, LC], F32, tag="rden")
                    nc.vector.reciprocal_approx_fast(rden, d1b)
                    crt = mid.tile([128, LC], F32, tag="crt")
                    cr_eng = nc.gpsimd if cr_on_pool else nc.vector
                    cr_eng.tensor_mul(crt, f32v(ct), rden)
                    s1t = expst
                else:
                    # S1T = expST / d1b
                    r1b = mid.tile([LQ, LC], F32, tag="r1b")
                    nc.vector.reciprocal_approx_fast(r1b, d1b)
                    s1t = mid.tile([LQ, LC], FR_BIG, tag="s1t")
                    s1t_eng = nc.gpsimd if s1t_on_pool else nc.vector
                    s1t_eng.tensor_mul(s1t, f32v(expst), r1b)
                    rden = crt = None

                # merged PSUM->SBUF copies: pp_a -> sba (ctT 0-2 + qt^T),
                # pp_b[0:328] -> sbb (expS^T + ctT3)
                sba = mid.tile([128, 512], b_t1dt, tag="sba")
                if b >= bpc - sba_dve_tail:
                    nc.vector.tensor_copy(sba, pa)
                else:
                    nc.scalar.copy(sba, pa)
                sbb = mid.tile([128, 328], b_t1dt, tag="sbb")
                if b >= bpc - sbb_dve_tail:
                    nc.vector.tensor_copy(sbb, pb_t[:, :328])
                else:
                    nc.scalar.copy(sbb, pb_t[:, :328])
                if b_t1t_bf16:
                    qt_sb = mid.tile([LQ, 128], FR_BIG, tag="qtsb")
                    nc.scalar.copy(qt_sb, pa[:LQ, LC3:])
                    qt_l = qt_sb[:]
                else:
                    qt_l = sba[:LQ, LC3:]
                stash[b] = (ct, s1t, sba, sbb, pb_t, r2, qt_l, rden, crt)

            def back(b):
                (ct, s1t, sba, sbb, pb_t, r2, qt_l, rden, crt) = stash.pop(b)

                if cstore_back and b >= d2d_cstore:
                    # C passthrough store: its load finished long ago, so
                    # this never stalls the queue head
                    cs = {"sp": nc.sync, "act": nc.scalar,
                          "pool": nc.gpsimd}[cstore_on]
                    cs.dma_start(out_ap[b, 0:D, :], f32v(ct))

                abt = pp_ab.tile([D, 1024], F32, tag="ab")
                a_ps = abt[:, 0:LC]
                bm_ps = abt[:, 512:512 + LC]
                if a_first:
                    # A only needs qt/expST: issue it ahead of the t1t chain
                    # so o1/o2 and the split store flow while t1t/Bm compute
                    nc.tensor.matmul(a_ps, qt_l, s1t, start=True, stop=True)

                # T1T_raw = sum_c expS_c^T @ CT_c  [50,128]
                t1tp = pb_t[:LQ, 328:456]
                for c in range(3):
                    nc.tensor.matmul(
                        t1tp,
                        sbb[:, c * LQ:(c + 1) * LQ],
                        sba[:, c * 128:(c + 1) * 128],
                        start=(c == 0), stop=False)
                nc.tensor.matmul(
                    t1tp,
                    sbb[:LCR, 3 * LQ:4 * LQ],
                    sbb[:LCR, 200:328],
                    start=False, stop=True)
                t1t_sb = mid.tile([LQ, D], FR_BIG, tag="t1tsb")
                if b >= bpc - t1tdve_tail:
                    nc.vector.tensor_scalar(t1t_sb, t1tp, r2, None, ALU.mult)
                else:
                    nc.scalar.mul(t1t_sb, t1tp, r2)

                # A = QT^T @ S1T ; Bm = T1T^T @ S1T  [128,400]
                if not a_first:
                    nc.tensor.matmul(a_ps, qt_l, s1t, start=True, stop=True)
                nc.tensor.matmul(bm_ps, t1t_sb, s1t, start=True, stop=True)

                # outputs: o1|o2|o3 packed for a single merged store
                outbuf = outp.tile([D, 3 * LC], F32, tag="o")
                cmul = crt[:] if fold_den1 else f32v(ct)
                b_split_o23 = split_o23 or (b >= bpc - o23_tail)
                b_o1_act = o1_on_act or (b >= bpc - o1act_tail)
                if fold_den1 and b_o1_act:
                    o1sb = mid.tile([D, LC], F32, tag="o1sb")
                    nc.scalar.copy(o1sb, a_ps)
                    nc.gpsimd.tensor_mul(outbuf[:, :LC], rden, o1sb)
                elif fold_den1:
                    # o1 = A_u * R (the deferred softmax-over-j normalize)
                    nc.vector.tensor_mul(outbuf[:, :LC], rden, a_ps)
                elif o1_split:
                    nc.vector.tensor_copy(outbuf[:, :LC // 2],
                                          a_ps[:, :LC // 2])
                    nc.scalar.copy(outbuf[:, LC // 2:LC], a_ps[:, LC // 2:])
                elif o1_on_act:
                    nc.scalar.copy(outbuf[:, :LC], a_ps)
                else:
                    nc.vector.tensor_copy(outbuf[:, :LC], a_ps)
                if b_split_o23:
                    # separate muls: o2 can start right after A while Bm is
                    # still in the PE; shortens the Bm->store tail
                    nc.vector.tensor_mul(outbuf[:, LC:2 * LC], cmul, a_ps)
                    nc.vector.tensor_mul(outbuf[:, 2 * LC:], cmul, bm_ps)
                else:
                    # one DVE pass: [C*A | C*Bm]; cmul free-dim broadcast
                    # over the two PSUM banks
                    nc.vector.tensor_mul(
                        outbuf[:, LC:].rearrange("p (t s) -> p t s", t=2),
                        cmul.unsqueeze(1).broadcast_to([D, 2, LC]),
                        abt[:].rearrange("p (t s) -> p t s", t=2)[:, :, :LC])

                if b >= bpc - split_store:
                    # tail batches: store [o1|o2] as soon as o2 lands, then
                    # o3 separately — shortens the drain chain
                    nc.sync.dma_start(
                        out_ap[b, D:3 * D, :].rearrange(
                            "(t d) i -> d t i", t=2),
                        outbuf[:, :2 * LC].rearrange(
                            "p (t s) -> p t s", t=2))
                    nc.sync.dma_start(out_ap[b, 3 * D:, :],
                                      outbuf[:, 2 * LC:])
                else:
                    nc.sync.dma_start(
                        out_ap[b, D:, :].rearrange("(t d) i -> d t i", t=3),
                        outbuf[:].rearrange("p (t s) -> p t s", t=3))

            lag = pipe_depth - 1
            for i in range(bpc + lag):
                if i < bpc:
                    front(i)
                if i >= lag:
                    back(i - lag)

    nc.compile()
    return nc


_NC_CACHE = {}
last_exec_s = None
BUILD_KW = {"loads_on": "pool", "cr_on_pool": True, "fold_den1": True,
            "tail_fast": 9, "head_fast": 1, "warm_sp": True,
            "es_f32r": True, "o23_tail": 1, "split_store": 4,
            "stp_tail": 6, "t1t_tail": 6}


def _get_nc():
    key = tuple(sorted(BUILD_KW.items()))
    if key not in _NC_CACHE:
        _NC_CACHE[key] = build_nc(**BUILD_KW)
    return _NC_CACHE[key]


_EXEC_CACHE = {}


def _get_exec():
    """Build (once) a cached sharded PJRT callable for the kernel NEFF.

    Mirrors concourse.bass2jax.run_bass_via_pjrt's multi-core path, but
    caches the jitted function across calls and creates the donated
    output zero-buffers on-device (no 100MB host->device transfer of
    zeros per invocation).
    """
    if "fn" in _EXEC_CACHE:
        return _EXEC_CACHE
    import jax
    from jax.sharding import Mesh, PartitionSpec
    from jax.experimental.shard_map import shard_map
    from concourse import bass2jax, mybir
    from concourse.bass2jax import _bass_exec_p, partition_id_tensor

    bass2jax.install_neuronx_cc_hook()
    nc = _get_nc()

    partition_name = (nc.partition_id_tensor.name
                      if nc.partition_id_tensor else None)
    in_names, out_names, out_avals = [], [], []
    for alloc in nc.m.functions[0].allocations:
        if not isinstance(alloc, mybir.MemoryLocationSet):
            continue
        name = alloc.memorylocations[0].name
        if alloc.kind == "ExternalInput":
            if name != partition_name:
                in_names.append(name)
        elif alloc.kind == "ExternalOutput":
            out_names.append(name)
            out_avals.append(jax.core.ShapedArray(
                tuple(alloc.tensor_shape), mybir.dt.np(alloc.dtype)))
    n_params = len(in_names)
    all_in_names = list(in_names) + list(out_names)
    if partition_name is not None:
        all_in_names.append(partition_name)

    def _body(*args):
        operands = list(args)
        if partition_name is not None:
            operands.append(partition_id_tensor())
        outs = _bass_exec_p.bind(
            *operands,
            out_avals=tuple(out_avals),
            in_names=tuple(all_in_names),
            out_names=tuple(out_names),
            lowering_input_output_aliases=(),
            sim_require_finite=True,
            sim_require_nnan=True,
            nc=nc,
        )
        return tuple(outs)

    try:
        devices = jax.devices("axon")[:N_CORES]
    except Exception:
        devices = jax.devices()[:N_CORES]
    assert len(devices) >= N_CORES, f"need {N_CORES} cores, got {devices}"
    mesh = Mesh(np.asarray(devices[:N_CORES]), ("core",))
    n_outs = len(out_avals)
    donate = tuple(range(n_params, n_params + n_outs))
    in_specs = (PartitionSpec("core"),) * (n_params + n_outs)
    out_specs = (PartitionSpec("core"),) * n_outs
    fn = jax.jit(
        shard_map(_body, mesh=mesh, in_specs=in_specs, out_specs=out_specs,
                  check_rep=False),
        donate_argnums=donate, keep_unused=True)

    from jax.sharding import NamedSharding
    zero_shardings = [NamedSharding(mesh, PartitionSpec("core"))] * n_outs
    zero_shapes = [(N_CORES * a.shape[0], *a.shape[1:]) for a in out_avals]
    zero_dtypes = [a.dtype for a in out_avals]

    import jax.numpy as jnp
    make_zeros = jax.jit(
        lambda: tuple(jnp.zeros(s, d) for s, d in
                      zip(zero_shapes, zero_dtypes)),
        out_shardings=tuple(zero_shardings))

    _EXEC_CACHE.update(dict(fn=fn, in_names=in_names, out_names=out_names,
                            out_avals=out_avals, make_zeros=make_zeros,
                            mesh=mesh))
    return _EXEC_CACHE


def kernel(C, Q, W):
    global last_exec_s
    C = np.ascontiguousarray(C, dtype=np.float32)
    Q = np.ascontiguousarray(Q, dtype=np.float32)
    W = np.ascontiguousarray(W, dtype=np.float32)
    assert C.shape == (B, D, LC) and Q.shape == (B, D, LQ)
    assert W.shape == (B, 1, 3 * D)

    ex = _get_exec()
    full = {"C": C, "Q": Q, "W": W}
    ins = [full[n] for n in ex["in_names"]]
    t0 = time.monotonic()
    zeros = ex["make_zeros"]()
    out_arrs = ex["fn"](*ins, *zeros)
    out_arrs = [np.asarray(o) for o in out_arrs]
    last_exec_s = time.monotonic() - t0
    (oidx,) = [i for i, n in enumerate(ex["out_names"]) if n == "out"]
    return out_arrs[oidx].reshape(B, 4 * D, LC)


# revision 63
# speedup vs baseline: 1.0028x; 1.0028x over previous
"""CQAttention Trainium2 Bass kernel.

Computes, per batch b (B=128, D=128, LC=400, LQ=50):
    S = Wc.C (over rows) + Wq.Q (over cols) + Wqc.(C*Q)   [LC, LQ]
    S1 = softmax(S, axis=LQ); S2 = softmax(S, axis=LC)
    A  = Q @ S1^T                    [D, LC]
    Bm = (C @ S2) @ S1^T             [D, LC]
    out = concat([C, A, C*A, C*Bm])  [4D, LC]

Sharding: data-parallel over batch, 16 batches per core x 8 cores.

Kernel math (per batch, ST = S^T layout [LQ=50 part, LC=400 free]):
    QW[d,j]  = Wqc[d]*Q[d,j] + Wc[d]                (2-scalar op)
    ST'      = QW^T @ C                 [50,400]    (folds the Wc.C row term)
    cT[j]    = Q^T @ Wq                 [50,1]
    expST    = exp(ST' + cT)  (+row sums den2)      (ACT, accum_out)
    d1b      = ones[50,128]^T @ expST   [128,400]   (bcast column sums over j)
    R        = recip(d1b); CR = C*R                 (softmax-over-j deferred
               to the output muls: keeps d1b/recip off the critical chain)
    expS/CT  = PE transposes of expST / C; chunks 0-2 full 128-wide, chunk 3
               is the 16 real trailing rows (no zero padding anywhere)
    T1T_raw  = sum_c expS_c^T @ CT_c    [50,128]
    T1T      = T1T_raw * recip(den2)                (softmax over i, folded)
    A_u      = QT^T @ expST             [128,400]   (unnormalized)
    Bm_u     = T1T^T @ expST            [128,400]
    out rows: [C, R*A_u, CR*A_u, CR*Bm_u]

Schedule: two-stage software pipeline (FRONT(b+1) issues before BACK(b))
so exp(b+1) stays ahead of t1t_sb(b) in the in-order ACT queue; C/Q loads
issue on the Pool SWDGE queue (no HWDGE contention, no head-of-line
blocking behind waiting stores); stores on SP. C/Q rings are f32r-typed
so edge batches (head_fast/tail_fast), which run at mid p-state, can use
f32r matmuls/transposes and a bf16 T1 path to halve the latency chain
where the PE is not streaming.

PSUM layout (8 banks, all pools depth 2):
    pp_st  x2: [50,404]  stp(0:400)+ctp(400:401); d1b overwrites stp after exp
    pp_a   x2: [128,512] ctT chunks 0-2 (0:384) + qtp (50p, 384:512)
    pp_b   x2: [128,456] esp (0:200) + ctT3 (16p, 200:328) + t1tp (50p, 328:456)
    pp_ab  x1: [128,1024] A (0:400) + Bm (512:912), two banks
"""

import os
import sys
import time

# The kernel executes via the axon PJRT backend; make sure it isn't masked
# by an explicit cpu pin (harmless if jax is already initialized with axon).
_jp = os.environ.get("JAX_PLATFORMS", "")
if _jp and "axon" not in _jp:
    os.environ["JAX_PLATFORMS"] = "axon," + _jp

for _p in ("/opt/trn_rl_repo", "/root/.axon_site/_ro/trn_rl_repo"):
    if _p not in sys.path:
        sys.path.append(_p)

import numpy as np

B, D, LC, LQ = 128, 128, 400, 50
N_CORES = 8
BPC = B // N_CORES  # 16 batches per core
LC3 = 3 * 128       # columns covered by the three full transpose chunks
LCR = LC - LC3      # 16 trailing rows handled by the narrow chunk

USE_F32R_BIG = True


def build_nc(bpc=BPC, use_f32r_big=USE_F32R_BIG, enable_asserts=False,
             mid_bufs=7, outp_bufs=6, io_bufs=5, qw_pool_engine=True,
             c_halves=5, e_slots=5, ab_bufs=1, st_bufs=2, a_bufs=2, b_bufs=2,
             t1t_bf16=False, stp_f32r=False, tr_f32r=False,
             s1t_on_pool=False, o1_on_act=False, o1_split=False,
             loads_on_pool=False, load_group=2, split_o23=False,
             split_store=0, loads_on="sp", cstore_on="sp", cstore_back=True,
             warm_sp=False, pipe_depth=2, fold_den1=True, cr_on_pool=True,
             head_fast=0, tail_fast=99, o23_tail=0, o1act_tail=0,
             es_f32r=False, c_f32r=True, stp_tail=99, w_on_act=False,
             t1tdve_tail=0, warm_act=False, d2d_cstore=0, d2d_on_sp=False,
             a_first=False, sbb_dve_tail=0, sba_dve_tail=0,
             tr_tail=99, t1t_tail=99, stp_head=0, t1t_head=0,
             pool_alloc_mode="stack", detect_races=True):
    import concourse.bacc as bacc
    import concourse.tile as tile
    from concourse import mybir
    from concourse.masks import make_identity

    F32 = mybir.dt.float32
    F32R = mybir.dt.float32r
    BF16 = mybir.dt.bfloat16
    AFT = mybir.ActivationFunctionType
    ALU = mybir.AluOpType

    FR_BIG = F32R if use_f32r_big else F32
    T1DT = BF16 if t1t_bf16 else F32

    assert bpc % 2 == 0
    assert load_group == 2
    nc = bacc.Bacc("TRN2", target_bir_lowering=False, debug=False,
                   enable_asserts=enable_asserts, num_devices=N_CORES,
                   detect_race_conditions=detect_races)
    C_ap = nc.dram_tensor("C", [bpc, D, LC], F32, kind="ExternalInput").ap()
    Q_ap = nc.dram_tensor("Q", [bpc, D, LQ], F32, kind="ExternalInput").ap()
    W_ap = nc.dram_tensor("W", [bpc, 1, 3 * D], F32, kind="ExternalInput").ap()
    out_ap = nc.dram_tensor("out", [bpc, 4 * D, LC], F32,
                            kind="ExternalOutput").ap()

    with tile.TileContext(nc, pool_alloc_mode=pool_alloc_mode) as tc:
        from contextlib import ExitStack
        with ExitStack() as ctx:
            consts = ctx.enter_context(tc.tile_pool(name="consts", bufs=1))
            io = ctx.enter_context(tc.tile_pool(name="io", bufs=io_bufs))
            mid = ctx.enter_context(tc.tile_pool(name="mid", bufs=mid_bufs))
            outp = ctx.enter_context(tc.tile_pool(name="outp", bufs=outp_bufs))
            pp_st = ctx.enter_context(
                tc.tile_pool(name="pp_st", bufs=st_bufs, space="PSUM"))
            pp_a = ctx.enter_context(
                tc.tile_pool(name="pp_a", bufs=a_bufs, space="PSUM"))
            pp_b = ctx.enter_context(
                tc.tile_pool(name="pp_b", bufs=b_bufs, space="PSUM"))
            pp_ab = ctx.enter_context(
                tc.tile_pool(name="pp_ab", bufs=ab_bufs, space="PSUM"))

            # --- constants ---
            ident = consts.tile([128, 128], F32)
            make_identity(nc, ident)
            ident_r_t = consts.tile([128, 128], F32R)
            nc.vector.tensor_copy(ident_r_t, ident)
            ident_r = ident_r_t[:]
            ones_f32 = consts.tile([LQ, 128], F32)
            nc.vector.memset(ones_f32, 1.0)
            onesmat = consts.tile([LQ, 128], FR_BIG)
            nc.vector.tensor_copy(onesmat, ones_f32)

            # --- W preload: [bpc,384] -> per-d columns [128, 3*bpc] ---
            w_stage = consts.tile([bpc, 3 * D], F32)
            (nc.scalar if w_on_act else nc.sync).dma_start(
                w_stage, W_ap[:, 0, :])
            wTp = pp_a.tile([128, 3 * bpc], F32, tag="a")
            for k in range(3):
                nc.tensor.matmul(
                    wTp[:, k * bpc:(k + 1) * bpc],
                    w_stage[:, k * D:(k + 1) * D],
                    ident[:bpc, :bpc],
                    is_transpose=True, start=True, stop=True)
            w_all = consts.tile([128, 3 * bpc], F32)
            nc.vector.tensor_copy(w_all, wTp)

            # C passthrough rows for the first few batches go as cold
            # DRAM->DRAM copies on the ACT queue: they have no deps, so the
            # transfer lands in the startup hole of the DMA stream.
            for _b in range(d2d_cstore):
                (nc.sync if d2d_on_sp else nc.scalar).dma_start(
                    out_ap[_b, 0:D, :], C_ap[_b, :, :])

            # Manually double-buffered C-pair and expST rings (no pad columns
            # needed: the 4th transpose chunk is 16 partitions wide).
            CDT = F32R if c_f32r else F32
            cbuf = consts.tile([D, c_halves * 2 * LC], CDT)
            ebuf_f32 = consts.tile([LQ, e_slots * LC], F32)
            ebuf = ebuf_f32[:].bitcast(FR_BIG) if FR_BIG != F32 else ebuf_f32[:]

            def f32v(ap):
                return ap.bitcast(F32) if ap.dtype == F32R else ap

            # Two-stage software pipeline: FRONT(b) produces everything up
            # to the merged SBUF copies; BACK(b) (issued after FRONT(b+1))
            # runs t1t/A/Bm/output ops. This keeps exp(b+1) ahead of
            # t1t_sb(b) in the ACT queue, breaking the per-batch
            # ACT->PE->ACT round-trip that otherwise sets the initiation
            # interval.
            qpair = None
            stash = {}

            def front(b):
                nonlocal qpair
                # edge batches run at mid p-state (PE not continuously
                # busy), where f32r/bf16 matmuls halve the latency chain
                fast = (b < head_fast) or (b >= tail_fast)
                b_stp_f32r = (stp_f32r or fast or b >= stp_tail
                              or b < stp_head) and c_f32r
                b_tr_f32r = (fast or b >= tr_tail) and c_f32r
                b_es_f32r = (es_f32r or fast) and FR_BIG == F32R
                b_t1t_bf16 = (t1t_bf16 or fast or b >= t1t_tail
                              or b < t1t_head)
                b_t1dt = BF16 if b_t1t_bf16 else F32R
                wq_col = w_all[:, b:b + 1]
                wc_col = w_all[:, bpc + b:bpc + b + 1]
                wqc_col = w_all[:, 2 * bpc + b:2 * bpc + b + 1]

                k = b % 2
                half = (b // 2) % c_halves
                cpair = cbuf[:, half * 2 * LC:(half + 1) * 2 * LC]
                if b % load_group == 0:
                    g = load_group
                    ld = {"sp": nc.sync, "act": nc.scalar,
                          "pool": nc.gpsimd}[loads_on]
                    if loads_on_pool:
                        ld = nc.gpsimd
                    dst = cbuf[:, half * 2 * LC:half * 2 * LC + g * LC]
                    qpair = io.tile([D, g * LQ], CDT, tag="qpair")
                    if b == 0 and warm_sp:
                        # cold start: single-batch C first on the fast HWDGE
                        # path so exp(0) fires as early as possible; with
                        # warm_act, C0/Q0 issue on the ACT queue in parallel
                        # with the W load on SP
                        w0 = nc.scalar if warm_act else nc.sync
                        w0.dma_start(dst[:, :LC],
                                     C_ap[0, :, :].bitcast(CDT))
                        w0.dma_start(
                            qpair[:].rearrange("p (t s) -> p t s", t=g),
                            Q_ap[0:g].rearrange("t d j -> d t j").bitcast(CDT))
                        nc.sync.dma_start(dst[:, LC:2 * LC],
                                          C_ap[1, :, :].bitcast(CDT))
                    else:
                        ld.dma_start(
                            dst.rearrange("p (t s) -> p t s", t=g),
                            C_ap[b:b + g].rearrange("t d i -> d t i").bitcast(CDT))
                        ld.dma_start(
                            qpair[:].rearrange("p (t s) -> p t s", t=g),
                            Q_ap[b:b + g].rearrange("t d j -> d t j").bitcast(CDT))
                ct = cpair[:, k * LC:(k + 1) * LC]       # [128, 400]
                qt = qpair[:, (b % load_group) * LQ:
                           (b % load_group + 1) * LQ]    # [128, 50]

                if not cstore_back:
                    nc.sync.dma_start(out_ap[b, 0:D, :], f32v(ct))

                # QW = Wqc*Q + Wc  (two-scalar op)
                qw = mid.tile([D, LQ], F32R if b_stp_f32r else F32,
                              tag="qw")
                qw_eng = nc.gpsimd if qw_pool_engine else nc.vector
                qw_eng.tensor_scalar(qw, f32v(qt), wqc_col, wc_col,
                                     ALU.mult, ALU.add)

                # ST' = QW^T @ C [50,400]; cT = Q^T @ Wq packed into the
                # spare bytes of the same PSUM bank
                stb = pp_st.tile([128, LC + 4], F32, tag="st", name="stb")
                stp = stb[:LQ, :LC]
                if b_stp_f32r:
                    nc.tensor.matmul(stp, qw, ct, start=True, stop=True)
                else:
                    nc.tensor.matmul(stp, qw, f32v(ct),
                                     start=True, stop=True)
                ctp = stb[:LQ, LC:LC + 1]
                nc.tensor.matmul(ctp, f32v(qt), wq_col,
                                 start=True, stop=True)
                ct_sb = mid.tile([LQ, 1], F32, tag="ctsb")
                nc.vector.tensor_copy(ct_sb, ctp)

                # QT transpose [50,128] into pp_a cols 384:512; C^T chunks
                # 0-2 into cols 0:384; the 16-row chunk 3 into pp_b.
                pa = pp_a.tile([128, 512], F32, tag="a", name="pa")
                pb_t = pp_b.tile([128, 456], F32, tag="b", name="pb")
                if b_tr_f32r:
                    nc.tensor.matmul(pa[:LQ, LC3:].bitcast(F32R),
                                     qt, ident_r,
                                     is_transpose=True, start=True, stop=True)
                    for c in range(3):
                        nc.tensor.matmul(
                            pa[:, c * 128:(c + 1) * 128].bitcast(F32R),
                            ct[:, c * 128:(c + 1) * 128],
                            ident_r, is_transpose=True, start=True, stop=True)
                    nc.tensor.matmul(
                        pb_t[:LCR, 200:328].bitcast(F32R),
                        ct[:, LC3:], ident_r,
                        is_transpose=True, start=True, stop=True)
                else:
                    nc.tensor.matmul(pa[:LQ, LC3:], f32v(qt), ident,
                                     is_transpose=True, start=True, stop=True)
                    for c in range(3):
                        nc.tensor.matmul(
                            pa[:, c * 128:(c + 1) * 128],
                            f32v(ct[:, c * 128:(c + 1) * 128]), ident,
                            is_transpose=True, start=True, stop=True)
                    nc.tensor.matmul(pb_t[:LCR, 200:328], f32v(ct[:, LC3:]),
                                     ident,
                                     is_transpose=True, start=True, stop=True)

                # expST = exp(ST' + cT), den2 = row sums
                eslot = b % e_slots
                expst = ebuf[:, eslot * LC:(eslot + 1) * LC]
                den2 = mid.tile([LQ, 1], F32, tag="den2")
                nc.scalar.activation(expst, stp, AFT.Exp, bias=ct_sb,
                                     accum_out=den2)
                r2 = mid.tile([LQ, 1], F32, tag="r2")
                nc.vector.reciprocal_approx_fast(r2, den2)

                # den1 broadcast: d1b[p,i] = sum_j expST[j,i]; overwrites stp
                # (exp is done). With fold_den1 the broadcast covers all 128
                # partitions so the recip can scale the A/Bm outputs directly.
                if fold_den1:
                    d1b = stb[:, :LC]
                    nc.tensor.matmul(d1b, onesmat, expst,
                                     start=True, stop=True)
                else:
                    d1b = stb[:LQ, :LC]
                    nc.tensor.matmul(d1b, onesmat[:, :LQ], expst,
                                     start=True, stop=True)

                # expS^T chunks into pp_b cols 0:200 (chunk 3 is 16 rows)
                if b_es_f32r:
                    for c in range(3):
                        es_in = expst[:, c * 128:(c + 1) * 128]
                        if es_in.dtype != F32R:
                            es_in = es_in.bitcast(F32R)
                        nc.tensor.matmul(
                            pb_t[:, c * LQ:(c + 1) * LQ].bitcast(F32R),
                            es_in, ident_r[:LQ, :LQ],
                            is_transpose=True, start=True, stop=True)
                    es3 = expst[:, LC3:]
                    if es3.dtype != F32R:
                        es3 = es3.bitcast(F32R)
                    nc.tensor.matmul(pb_t[:LCR, 3 * LQ:4 * LQ].bitcast(F32R),
                                     es3, ident_r[:LQ, :LQ],
                                     is_transpose=True, start=True, stop=True)
                else:
                    for c in range(3):
                        nc.tensor.matmul(
                            pb_t[:, c * LQ:(c + 1) * LQ],
                            f32v(expst[:, c * 128:(c + 1) * 128]),
                            ident[:LQ, :LQ],
                            is_transpose=True, start=True, stop=True)
                    nc.tensor.matmul(pb_t[:LCR, 3 * LQ:4 * LQ],
                                     f32v(expst[:, LC3:]), ident[:LQ, :LQ],
                                     is_transpose=True, start=True, stop=True)

                if fold_den1:
                    # R = 1/den1 over all 128 partitions; CR = C*R. The A/Bm
                    # matmuls then use raw expST as rhs (softmax-over-j is
                    # applied at the output muls), which drops d1b->r1b->s1t
                    # from the inter-batch critical chain.
                    rden = mid.tile([128, LC], F32, tag="rden")
                    nc.vector.reciprocal_approx_fast(rden, d1b)
                    crt = mid.tile([128, LC], F32, tag="crt")
                    cr_eng = nc.gpsimd if cr_on_pool else nc.vector
                    cr_eng.tensor_mul(crt, f32v(ct), rden)
                    s1t = expst
                else:
                    # S1T = expST / d1b
                    r1b = mid.tile([LQ, LC], F32, tag="r1b")
                    nc.vector.reciprocal_approx_fast(r1b, d1b)
                    s1t = mid.tile([LQ, LC], FR_BIG, tag="s1t")
                    s1t_eng = nc.gpsimd if s1t_on_pool else nc.vector
                    s1t_eng.tensor_mul(s1t, f32v(expst), r1b)
                    rden = crt = None

                # merged PSUM->SBUF copies: pp_a -> sba (ctT 0-2 + qt^T),
                # pp_b[0:328] -> sbb (expS^T + ctT3)
                sba = mid.tile([128, 512], b_t1dt, tag="sba")
                if b >= bpc - sba_dve_tail:
                    nc.vector.tensor_copy(sba, pa)
                else:
                    nc.scalar.copy(sba, pa)
                sbb = mid.tile([128, 328], b_t1dt, tag="sbb")
                if b >= bpc - sbb_dve_tail:
                    nc.vector.tensor_copy(sbb, pb_t[:, :328])
                else:
                    nc.scalar.copy(sbb, pb_t[:, :328])
                if b_t1t_bf16:
                    qt_sb = mid.tile([LQ, 128], FR_BIG, tag="qtsb")
                    nc.scalar.copy(qt_sb, pa[:LQ, LC3:])
                    qt_l = qt_sb[:]
                else:
                    qt_l = sba[:LQ, LC3:]
                stash[b] = (ct, s1t, sba, sbb, pb_t, r2, qt_l, rden, crt)

            def back(b):
                (ct, s1t, sba, sbb, pb_t, r2, qt_l, rden, crt) = stash.pop(b)

                if cstore_back and b >= d2d_cstore:
                    # C passthrough store: its load finished long ago, so
                    # this never stalls the queue head
                    cs = {"sp": nc.sync, "act": nc.scalar,
                          "pool": nc.gpsimd}[cstore_on]
                    cs.dma_start(out_ap[b, 0:D, :], f32v(ct))

                abt = pp_ab.tile([D, 1024], F32, tag="ab")
                a_ps = abt[:, 0:LC]
                bm_ps = abt[:, 512:512 + LC]
                if a_first:
                    # A only needs qt/expST: issue it ahead of the t1t chain
                    # so o1/o2 and the split store flow while t1t/Bm compute
                    nc.tensor.matmul(a_ps, qt_l, s1t, start=True, stop=True)

                # T1T_raw = sum_c expS_c^T @ CT_c  [50,128]
                t1tp = pb_t[:LQ, 328:456]
                for c in range(3):
                    nc.tensor.matmul(
                        t1tp,
                        sbb[:, c * LQ:(c + 1) * LQ],
                        sba[:, c * 128:(c + 1) * 128],
                        start=(c == 0), stop=False)
                nc.tensor.matmul(
                    t1tp,
                    sbb[:LCR, 3 * LQ:4 * LQ],
                    sbb[:LCR, 200:328],
                    start=False, stop=True)
                t1t_sb = mid.tile([LQ, D], FR_BIG, tag="t1tsb")
                if b >= bpc - t1tdve_tail:
                    nc.vector.tensor_scalar(t1t_sb, t1tp, r2, None, ALU.mult)
                else:
                    nc.scalar.mul(t1t_sb, t1tp, r2)

                # A = QT^T @ S1T ; Bm = T1T^T @ S1T  [128,400]
                if not a_first:
                    nc.tensor.matmul(a_ps, qt_l, s1t, start=True, stop=True)
                nc.tensor.matmul(bm_ps, t1t_sb, s1t, start=True, stop=True)

                # outputs: o1|o2|o3 packed for a single merged store
                outbuf = outp.tile([D, 3 * LC], F32, tag="o")
                cmul = crt[:] if fold_den1 else f32v(ct)
                b_split_o23 = split_o23 or (b >= bpc - o23_tail)
                b_o1_act = o1_on_act or (b >= bpc - o1act_tail)
                if fold_den1 and b_o1_act:
                    o1sb = mid.tile([D, LC], F32, tag="o1sb")
                    nc.scalar.copy(o1sb, a_ps)
                    nc.gpsimd.tensor_mul(outbuf[:, :LC], rden, o1sb)
                elif fold_den1:
                    # o1 = A_u * R (the deferred softmax-over-j normalize)
                    nc.vector.tensor_mul(outbuf[:, :LC], rden, a_ps)
                elif o1_split:
                    nc.vector.tensor_copy(outbuf[:, :LC // 2],
                                          a_ps[:, :LC // 2])
                    nc.scalar.copy(outbuf[:, LC // 2:LC], a_ps[:, LC // 2:])
                elif o1_on_act:
                    nc.scalar.copy(outbuf[:, :LC], a_ps)
                else:
                    nc.vector.tensor_copy(outbuf[:, :LC], a_ps)
                if b_split_o23:
                    # separate muls: o2 can start right after A while Bm is
                    # still in the PE; shortens the Bm->store tail
                    nc.vector.tensor_mul(outbuf[:, LC:2 * LC], cmul, a_ps)
                    nc.vector.tensor_mul(outbuf[:, 2 * LC:], cmul, bm_ps)
                else:
                    # one DVE pass: [C*A | C*Bm]; cmul free-dim broadcast
                    # over the two PSUM banks
                    nc.vector.tensor_mul(
                        outbuf[:, LC:].rearrange("p (t s) -> p t s", t=2),
                        cmul.unsqueeze(1).broadcast_to([D, 2, LC]),
                        abt[:].rearrange("p (t s) -> p t s", t=2)[:, :, :LC])

                if b >= bpc - split_store:
                    # tail batches: store [o1|o2] as soon as o2 lands, then
                    # o3 separately — shortens the drain chain
                    nc.sync.dma_start(
                        out_ap[b, D:3 * D, :].rearrange(
                            "(t d) i -> d t i", t=2),
                        outbuf[:, :2 * LC].rearrange(
                            "p (t s) -> p t s", t=2))
                    nc.sync.dma_start(out_ap[b, 3 * D:, :],
                                      outbuf[:, 2 * LC:])
                else:
                    nc.sync.dma_start(
                        out_ap[b, D:, :].rearrange("(t d) i -> d t i", t=3),
                        outbuf[:].rearrange("p (t s) -> p t s", t=3))

            lag = pipe_depth - 1
            for i in range(bpc + lag):
                if i < bpc:
                    front(i)
                if i >= lag:
                    back(i - lag)

    nc.compile()
    return nc


_NC_CACHE = {}
last_exec_s = None
BUILD_KW = {"loads_on": "pool", "cr_on_pool": True, "fold_den1": True,
            "tail_fast": 9, "head_fast": 1, "warm_sp": True,
            "es_f32r": True, "o23_tail": 1, "split_store": 4,
            "stp_tail": 6, "t1t_tail": 6, "mid_bufs": 6}


def _get_nc():
    key = tuple(sorted(BUILD_KW.items()))
    if key not in _NC_CACHE:
        _NC_CACHE[key] = build_nc(**BUILD_KW)
    return _NC_CACHE[key]


_EXEC_CACHE = {}


def _get_exec():
    """Build (once) a cached sharded PJRT callable for the kernel NEFF.

    Mirrors concourse.bass2jax.run_bass_via_pjrt's multi-core path, but
    caches the jitted function across calls and creates the donated
    output zero-buffers on-device (no 100MB host->device transfer of
    zeros per invocation).
    """
    if "fn" in _EXEC_CACHE:
        return _EXEC_CACHE
    import jax
    from jax.sharding import Mesh, PartitionSpec
    from jax.experimental.shard_map import shard_map
    from concourse import bass2jax, mybir
    from concourse.bass2jax import _bass_exec_p, partition_id_tensor

    bass2jax.install_neuronx_cc_hook()
    nc = _get_nc()

    partition_name = (nc.partition_id_tensor.name
                      if nc.partition_id_tensor else None)
    in_names, out_names, out_avals = [], [], []
    for alloc in nc.m.functions[0].allocations:
        if not isinstance(alloc, mybir.MemoryLocationSet):
            continue
        name = alloc.memorylocations[0].name
        if alloc.kind == "ExternalInput":
            if name != partition_name:
                in_names.append(name)
        elif alloc.kind == "ExternalOutput":
            out_names.append(name)
            out_avals.append(jax.core.ShapedArray(
                tuple(alloc.tensor_shape), mybir.dt.np(alloc.dtype)))
    n_params = len(in_names)
    all_in_names = list(in_names) + list(out_names)
    if partition_name is not None:
        all_in_names.append(partition_name)

    def _body(*args):
        operands = list(args)
        if partition_name is not None:
            operands.append(partition_id_tensor())
        outs = _bass_exec_p.bind(
            *operands,
            out_avals=tuple(out_avals),
            in_names=tuple(all_in_names),
            out_names=tuple(out_names),
            lowering_input_output_aliases=(),
            sim_require_finite=True,
            sim_require_nnan=True,
            nc=nc,
        )
        return tuple(outs)

    try:
        devices = jax.devices("axon")[:N_CORES]
    except Exception:
        devices = jax.devices()[:N_CORES]
    assert len(devices) >= N_CORES, f"need {N_CORES} cores, got {devices}"
    mesh = Mesh(np.asarray(devices[:N_CORES]), ("core",))
    n_outs = len(out_avals)
    donate = tuple(range(n_params, n_params + n_outs))
    in_specs = (PartitionSpec("core"),) * (n_params + n_outs)
    out_specs = (PartitionSpec("core"),) * n_outs
    fn = jax.jit(
        shard_map(_body, mesh=mesh, in_specs=in_specs, out_specs=out_specs,
                  check_rep=False),
        donate_argnums=donate, keep_unused=True)

    from jax.sharding import NamedSharding
    zero_shardings = [NamedSharding(mesh, PartitionSpec("core"))] * n_outs
    zero_shapes = [(N_CORES * a.shape[0], *a.shape[1:]) for a in out_avals]
    zero_dtypes = [a.dtype for a in out_avals]

    import jax.numpy as jnp
    make_zeros = jax.jit(
        lambda: tuple(jnp.zeros(s, d) for s, d in
                      zip(zero_shapes, zero_dtypes)),
        out_shardings=tuple(zero_shardings))

    _EXEC_CACHE.update(dict(fn=fn, in_names=in_names, out_names=out_names,
                            out_avals=out_avals, make_zeros=make_zeros,
                            mesh=mesh))
    return _EXEC_CACHE


def kernel(C, Q, W):
    global last_exec_s
    C = np.ascontiguousarray(C, dtype=np.float32)
    Q = np.ascontiguousarray(Q, dtype=np.float32)
    W = np.ascontiguousarray(W, dtype=np.float32)
    assert C.shape == (B, D, LC) and Q.shape == (B, D, LQ)
    assert W.shape == (B, 1, 3 * D)

    ex = _get_exec()
    full = {"C": C, "Q": Q, "W": W}
    ins = [full[n] for n in ex["in_names"]]
    t0 = time.monotonic()
    zeros = ex["make_zeros"]()
    out_arrs = ex["fn"](*ins, *zeros)
    out_arrs = [np.asarray(o) for o in out_arrs]
    last_exec_s = time.monotonic() - t0
    (oidx,) = [i for i, n in enumerate(ex["out_names"]) if n == "out"]
    return out_arrs[oidx].reshape(B, 4 * D, LC)
